# revision 3
# baseline (speedup 1.0000x reference)
"""CapsuleLayer (dynamic routing) Trainium2 kernel — PE-build redesign.

Problem: x [64, 2048, 8], W [1, 2048, 32, 16, 8] (f32)
  u_hat[b,i,o,j] = sum_d W[0,i,o,j,d] * x[b,i,d]
  3 routing iterations (softmax over o, weighted i-sum, squash, logit
  update), returns v [64, 32, 16].

Data-parallel over batch: 8 samples/core, W replicated, no collectives.

Per core:
  Build: u_hat via 128 TensorE matmuls. Group g = 16 capsules; stationary
  lhsT = block-diag x_bd[g] [(isub,d)=128, (isub,b)=128] bf16, moving
  rhs = W2[g] [(isub,d)=128, oj=512] bf16 -> PSUM [(isub,b), oj] f32.
  Cast-drain to SBUF bf16 (Vector/Scalar alternating), then per-g
  partition-major-flatten DMAs transpose into u_hat[q=(g%8)*16+isub,
  rt=g//8, b, oj] (i on partitions, routing layout).
  Routing: s_j = chat^T @ u_hat on PE (per b, K-accumulate over 16 rt
  tiles), diag extract via mask-mul + strided reduce, squash with fused
  tensor_tensor_reduce, v broadcast via K=1 matmul, logit update via
  bf16 tensor ops with a contiguous-halves add tree, softmax via Exp
  activation + strided reduce + per-b tensor_scalar.
"""

import sys

import numpy as np

sys.path.insert(0, "/opt/trn_rl_repo")

import ml_dtypes

import concourse.bacc as bacc
import concourse.mybir as mybir
from concourse import bass_utils
from concourse.tile import TileContext

F32 = mybir.dt.float32
BF16 = mybir.dt.bfloat16

N_CORES = 8
B, IN_CAPS, IN_DIM, OUT_CAPS, OUT_DIM = 64, 2048, 8, 32, 16
BC = B // N_CORES           # samples per core
OJ = OUT_CAPS * OUT_DIM     # 512
NG = IN_CAPS // 16          # 128 groups of 16 capsules
NRT = IN_CAPS // 128        # 16 routing i-tiles
NUM_ROUTING = 3
EPS = 1e-9

_CACHE: dict = {}


def build_nc(repeat=1, n_routing=NUM_ROUTING, do_update=True):
    mult = mybir.AluOpType.mult
    add = mybir.AluOpType.add
    AX = mybir.AxisListType.X
    Exp = mybir.ActivationFunctionType.Exp
    Sqrt = mybir.ActivationFunctionType.Sqrt

    nc = bacc.Bacc(
        "TRN2",
        target_bir_lowering=False,
        debug=False,
        enable_asserts=False,
        num_devices=1,
    )
    xbd_d = nc.dram_tensor("xbd", [NG, 128, 128], BF16, kind="ExternalInput")
    w2_d = nc.dram_tensor("w2", [NG, 128, OJ], BF16, kind="ExternalInput")
    mask_d = nc.dram_tensor("mask", [OUT_CAPS, OJ], F32, kind="ExternalInput")
    ones_d = nc.dram_tensor("ones", [1, 128], BF16, kind="ExternalInput")
    out_d = nc.dram_tensor("vout", [BC, OUT_CAPS, OUT_DIM], F32, kind="ExternalOutput")

    with TileContext(nc) as tc:
        with (
            tc.tile_pool(name="per", bufs=1) as per,
            tc.tile_pool(name="xp", bufs=2) as xp,
            tc.tile_pool(name="wp", bufs=2) as wp,
            tc.tile_pool(name="stg", bufs=2) as stg,
            tc.tile_pool(name="sm", bufs=2) as sm,
            tc.tile_pool(name="tp", bufs=1) as tp,
            tc.tile_pool(name="bps", bufs=2, space="PSUM") as bps,
            tc.tile_pool(name="gp", bufs=2, space="PSUM") as gp,
            tc.tile_pool(name="vp", bufs=1, space="PSUM") as vp,
        ):
            mask = per.tile([OUT_CAPS, OJ], F32, tag="mask")
            nc.sync.dma_start(mask[:], mask_d.ap())
            ones = per.tile([1, 128], BF16, tag="ones")
            nc.sync.dma_start(ones[:], ones_d.ap())
            cunif = per.tile([128, OUT_CAPS], BF16, tag="cunif")
            nc.vector.memset(cunif[:], 1.0 / OUT_CAPS)

            # big persistent tiles
            uhat = per.tile([128, NRT, BC, OJ], BF16, tag="uhat")     # 128 KB/p
            bij = per.tile([128, NRT, BC, OUT_CAPS], BF16, tag="bij")  # 8 KB/p
            chat = per.tile([128, NRT, BC, OUT_CAPS], BF16, tag="chat")
            vbc = per.tile([128, BC, OJ], BF16, tag="vbc")            # 8 KB/p

            for rep in range(repeat):
                # ---------------- build phase ----------------
                xbd_v = xbd_d.ap().rearrange("(qblk gg) p c -> qblk p gg c", gg=16)
                w2_v = w2_d.ap().rearrange("(hblk gg) p c -> hblk p gg c", gg=4)
                for blk in range(NRT):           # 16 blocks of 8 groups
                    stage = stg.tile([128, 8, OJ], BF16, tag="stage")
                    # batched loads: x_bd per 4 blocks, W2 per half-block
                    if blk % 2 == 0:
                        xg = xp.tile([128, 16, 128], BF16, tag="xg")
                        nc.sync.dma_start(xg[:], xbd_v[blk // 2])
                    for gg in range(8):
                        if gg % 4 == 0:
                            wg = wp.tile([128, 4, OJ], BF16, tag="wg")
                            eng = nc.scalar if (blk % 2 == 0) else nc.sync
                            eng.dma_start(wg[:], w2_v[blk * 2 + gg // 4])
                        if gg % 2 == 0:
                            ps = bps.tile([128, 2, OJ], F32, tag="ps")
                        nc.tensor.matmul(
                            ps[:, gg % 2, :], xg[:, (blk % 2) * 8 + gg, :], wg[:, gg % 4, :],
                            start=True, stop=True,
                        )
                        if gg % 2 == 1:
                            # paired cast-copy (2 banks), split Scalar/Vector
                            if gg % 4 == 1:
                                nc.scalar.copy(stage[:, gg - 1:gg + 1, :], ps[:])
                            else:
                                nc.vector.tensor_copy(
                                    stage[:, gg - 1:gg + 1, :], ps[:]
                                )
                    # transpose: stage[p=(isub*8+b), gg, oj] ->
                    #            uhat[q=gg*16+isub, blk, b, oj]
                    for gg in range(8):
                        eng = (nc.sync, nc.scalar, nc.gpsimd)[(blk * 8 + gg) % 3]
                        eng.dma_start(
                            uhat[gg * 16:(gg + 1) * 16, blk, :, :],
                            stage[:, gg, :],
                        )

                # ---------------- routing ----------------
                if n_routing == 0:
                    for b in range(BC):
                        nc.gpsimd.dma_start(
                            out_d.ap()[b],
                            uhat[0:OUT_CAPS, 0, b, 0:OUT_DIM],
                        )
                for r in range(n_routing):
                    for b in range(BC):
                        g_ps = gp.tile([OUT_CAPS, OJ], F32, tag="g_ps")
                        for rt in range(NRT):
                            lhsT = (
                                cunif[:]
                                if r == 0
                                else chat[:, rt, b, :]
                            )
                            nc.tensor.matmul(
                                g_ps[:],
                                lhsT,
                                uhat[:, rt, b, :],
                                start=(rt == 0),
                                stop=(rt == NRT - 1),
                            )
                        mprod = sm.tile([OUT_CAPS, OJ], BF16, tag="mprod")
                        nc.vector.tensor_mul(mprod[:], g_ps[:], mask[:])
                        s_b = sm.tile([OUT_CAPS, OUT_DIM], F32, tag="s_b")
                        nc.vector.tensor_reduce(
                            s_b[:],
                            mprod[:].rearrange("p (o j) -> p j o", o=OUT_CAPS),
                            axis=AX,
                            op=add,
                        )
                        # squash: v = s * s2 / ((1+s2) * sqrt(s2+eps))
                        sq = sm.tile([OUT_CAPS, OUT_DIM], F32, tag="sq")
                        s2 = sm.tile([OUT_CAPS, 1], F32, tag="s2")
                        nc.vector.tensor_mul(sq[:], s_b[:], s_b[:])
                        nc.vector.tensor_reduce(s2[:], sq[:], axis=AX, op=add)
                        t1 = sm.tile([OUT_CAPS, 1], F32, tag="t1")
                        nc.gpsimd.tensor_scalar_add(t1[:], s2[:], 1.0)
                        nc.vector.reciprocal(t1[:], t1[:])
                        t2 = sm.tile([OUT_CAPS, 1], F32, tag="t2")
                        nc.gpsimd.tensor_scalar_add(t2[:], s2[:], EPS)
                        nc.scalar.activation(t2[:], t2[:], Sqrt)
                        nc.vector.reciprocal(t2[:], t2[:])
                        nc.gpsimd.tensor_mul(t1[:], t1[:], t2[:])
                        nc.gpsimd.tensor_mul(t1[:], t1[:], s2[:])
                        v_b = sm.tile([OUT_CAPS, OUT_DIM], F32, tag="v_b")
                        nc.vector.tensor_scalar_mul(v_b[:], s_b[:], t1[:])
                        if r < n_routing - 1 and do_update:
                            # broadcast v_b to all 128 partitions (bf16)
                            vb16 = sm.tile([OUT_CAPS, OUT_DIM], BF16, tag="vb16")
                            nc.scalar.copy(vb16[:], v_b[:])
                            vrow = sm.tile([1, OJ], BF16, tag="vrow")
                            nc.sync.dma_start(vrow[:], vb16[:])
                            vps = vp.tile([128, OJ], F32, tag="vps")
                            nc.tensor.matmul(
                                vps[:], ones[:], vrow[:], start=True, stop=True
                            )
                            nc.scalar.copy(vbc[:, b, :], vps[:])
                        elif r == n_routing - 1:
                            nc.sync.dma_start(out_d.ap()[b], v_b[:])

                    if r < n_routing - 1 and do_update:
                        # logit update + softmax, per rt tile
                        for rt in range(NRT):
                            u4 = uhat[:, rt, :, :].rearrange(
                                "p b (o j) -> p b o j", o=OUT_CAPS
                            )
                            v4 = vbc[:].rearrange("p b (o j) -> p b o j", o=OUT_CAPS)
                            prod = tp.tile([128, BC, OUT_CAPS, 16], BF16, tag="prod")
                            nc.vector.tensor_mul(prod[:], u4, v4)
                            tr1 = tp.tile([128, BC, OUT_CAPS, 8], BF16, tag="tr1")
                            nc.vector.tensor_tensor(
                                tr1[:], prod[:, :, :, 0:8], prod[:, :, :, 8:16], op=add
                            )
                            tr2 = tp.tile([128, BC, OUT_CAPS, 4], BF16, tag="tr2")
                            nc.vector.tensor_tensor(
                                tr2[:], tr1[:, :, :, 0:4], tr1[:, :, :, 4:8], op=add
                            )
                            tr3 = tp.tile([128, BC, OUT_CAPS, 2], BF16, tag="tr3")
                            nc.vector.tensor_tensor(
                                tr3[:], tr2[:, :, :, 0:2], tr2[:, :, :, 2:4], op=add
                            )
                            t3a = tr3[:, :, :, 0:1].rearrange("p b o one -> p b (o one)")
                            t3b = tr3[:, :, :, 1:2].rearrange("p b o one -> p b (o one)")
                            if r == 0:
                                nc.gpsimd.tensor_tensor(
                                    bij[:, rt, :, :], t3a, t3b, op=add
                                )
                            else:
                                upd = tp.tile([128, BC, OUT_CAPS], BF16, tag="upd")
                                nc.gpsimd.tensor_tensor(upd[:], t3a, t3b, op=add)
                                nc.gpsimd.tensor_add(
                                    bij[:, rt, :, :], bij[:, rt, :, :], upd[:]
                                )
                            # softmax over o -> chat
                            ex = sm.tile([128, BC, OUT_CAPS], BF16, tag="ex")
                            nc.scalar.activation(ex[:], bij[:, rt, :, :], Exp)
                            zr = sm.tile([128, BC], F32, tag="zr")
                            nc.vector.tensor_reduce(zr[:], ex[:], axis=AX, op=add)
                            nc.vector.reciprocal(zr[:], zr[:])
                            for b in range(BC):
                                nc.gpsimd.tensor_scalar_mul(
                                    chat[:, rt, b, :], ex[:, b, :], zr[:, b:b + 1]
                                )
    nc.compile()
    return nc


def _prep_inputs(x: np.ndarray, W: np.ndarray):
    W0 = np.ascontiguousarray(W.reshape(IN_CAPS, OUT_CAPS, OUT_DIM, IN_DIM))
    # W2[g, isub*8+d, oj] = W0[g*16+isub, oj, d]
    w2 = (
        W0.reshape(NG, 16, OJ, IN_DIM)
        .transpose(0, 1, 3, 2)
        .reshape(NG, 128, OJ)
        .astype(ml_dtypes.bfloat16)
    )
    mask = np.zeros((OUT_CAPS, OJ), np.float32)
    for o in range(OUT_CAPS):
        mask[o, o * OUT_DIM:(o + 1) * OUT_DIM] = 1.0
    ones = np.ones((1, 128), ml_dtypes.bfloat16)

    in_maps = []
    idx = np.arange(16)
    for c in range(N_CORES):
        xc = x[c * BC:(c + 1) * BC]          # [8, 2048, 8]
        # A[g, isub, d, b] = xc[b, g*16+isub, d]
        A = xc.transpose(1, 2, 0).reshape(NG, 16, IN_DIM, BC)
        xbd_v = np.zeros((NG, 16, IN_DIM, 16, BC), np.float32)
        xbd_v[:, idx, :, idx, :] = A.transpose(1, 0, 2, 3)
        xbd = xbd_v.reshape(NG, 128, 128).astype(ml_dtypes.bfloat16)
        in_maps.append({"xbd": xbd, "w2": w2, "mask": mask, "ones": ones})
    return in_maps


def kernel(x: np.ndarray, W: np.ndarray) -> np.ndarray:
    x = np.asarray(x, dtype=np.float32)
    W = np.asarray(W, dtype=np.float32)
    if "nc" not in _CACHE:
        _CACHE["nc"] = build_nc()
    nc = _CACHE["nc"]
    in_maps = _prep_inputs(x, W)
    res = bass_utils.run_bass_kernel_spmd(nc, in_maps, core_ids=list(range(N_CORES)))
    out = np.concatenate([res.results[c]["vout"] for c in range(N_CORES)], axis=0)
    return out.astype(np.float32)


if __name__ == "__main__":
    xt = np.random.randn(B, IN_CAPS, IN_DIM).astype(np.float32)
    Wt = (np.random.randn(1, IN_CAPS, OUT_CAPS, OUT_DIM, IN_DIM) * 0.01).astype(
        np.float32
    )
    print(kernel(xt, Wt).shape)


# revision 4
# speedup vs baseline: 1.1359x; 1.1359x over previous
"""CapsuleLayer (dynamic routing) Trainium2 kernel — PE-build redesign.

Problem: x [64, 2048, 8], W [1, 2048, 32, 16, 8] (f32)
  u_hat[b,i,o,j] = sum_d W[0,i,o,j,d] * x[b,i,d]
  3 routing iterations (softmax over o, weighted i-sum, squash, logit
  update), returns v [64, 32, 16].

Data-parallel over batch: 8 samples/core, W replicated, no collectives.

Per core:
  Build: u_hat via 128 TensorE matmuls. Group g = 16 capsules; stationary
  lhsT = block-diag x_bd[g] [(isub,d)=128, (isub,b)=128] bf16, moving
  rhs = W2[g] [(isub,d)=128, oj=512] bf16 -> PSUM [(isub,b), oj] f32.
  Cast-drain to SBUF bf16 (Vector/Scalar alternating), then per-g
  partition-major-flatten DMAs transpose into u_hat[q=(g%8)*16+isub,
  rt=g//8, b, oj] (i on partitions, routing layout).
  Routing: s_j = chat^T @ u_hat on PE (per b, K-accumulate over 16 rt
  tiles), diag extract via mask-mul + strided reduce, squash with fused
  tensor_tensor_reduce, v broadcast via K=1 matmul, logit update via
  bf16 tensor ops with a contiguous-halves add tree, softmax via Exp
  activation + strided reduce + per-b tensor_scalar.
"""

import sys

import numpy as np

sys.path.insert(0, "/opt/trn_rl_repo")

import ml_dtypes

import concourse.bacc as bacc
import concourse.mybir as mybir
from concourse import bass_utils
from concourse.tile import TileContext

F32 = mybir.dt.float32
BF16 = mybir.dt.bfloat16

N_CORES = 8
B, IN_CAPS, IN_DIM, OUT_CAPS, OUT_DIM = 64, 2048, 8, 32, 16
BC = B // N_CORES           # samples per core
OJ = OUT_CAPS * OUT_DIM     # 512
NG = IN_CAPS // 16          # 128 groups of 16 capsules
NRT = IN_CAPS // 128        # 16 routing i-tiles
NUM_ROUTING = 3
EPS = 1e-9

_CACHE: dict = {}


def build_nc(repeat=1, n_routing=NUM_ROUTING, do_update=True):
    mult = mybir.AluOpType.mult
    add = mybir.AluOpType.add
    AX = mybir.AxisListType.X
    Exp = mybir.ActivationFunctionType.Exp
    Sqrt = mybir.ActivationFunctionType.Sqrt

    nc = bacc.Bacc(
        "TRN2",
        target_bir_lowering=False,
        debug=False,
        enable_asserts=False,
        num_devices=1,
    )
    xbd_d = nc.dram_tensor("xbd", [NG, 128, 128], BF16, kind="ExternalInput")
    w2_d = nc.dram_tensor("w2", [NG, 128, OJ], BF16, kind="ExternalInput")
    mask_d = nc.dram_tensor("mask", [OUT_CAPS, OJ], F32, kind="ExternalInput")
    ones_d = nc.dram_tensor("ones", [1, 128], BF16, kind="ExternalInput")
    out_d = nc.dram_tensor("vout", [BC, OUT_CAPS, OUT_DIM], F32, kind="ExternalOutput")

    with TileContext(nc) as tc:
        with (
            tc.tile_pool(name="per", bufs=1) as per,
            tc.tile_pool(name="xp", bufs=2) as xp,
            tc.tile_pool(name="wp", bufs=2) as wp,
            tc.tile_pool(name="stg", bufs=2) as stg,
            tc.tile_pool(name="sm", bufs=2) as sm,
            tc.tile_pool(name="tp", bufs=1) as tp,
            tc.tile_pool(name="bps", bufs=2, space="PSUM") as bps,
            tc.tile_pool(name="gp", bufs=2, space="PSUM") as gp,
            tc.tile_pool(name="vp", bufs=1, space="PSUM") as vp,
        ):
            mask = per.tile([OUT_CAPS, OJ], F32, tag="mask")
            nc.sync.dma_start(mask[:], mask_d.ap())
            ones = per.tile([1, 128], BF16, tag="ones")
            nc.sync.dma_start(ones[:], ones_d.ap())
            cunif = per.tile([128, OUT_CAPS], BF16, tag="cunif")
            nc.vector.memset(cunif[:], 1.0 / OUT_CAPS)

            # big persistent tiles
            uhat = per.tile([128, NRT, BC, OJ], BF16, tag="uhat")     # 128 KB/p
            bij = per.tile([128, NRT, BC, OUT_CAPS], BF16, tag="bij")  # 8 KB/p
            chat = per.tile([128, NRT, BC, OUT_CAPS], BF16, tag="chat")
            vbc = per.tile([128, BC, OJ], BF16, tag="vbc")            # 8 KB/p

            for rep in range(repeat):
                # ---------------- build phase ----------------
                xbd_v = xbd_d.ap().rearrange("(qblk gg) p c -> qblk p gg c", gg=16)
                w2_v = w2_d.ap().rearrange("(hblk gg) p c -> hblk p gg c", gg=4)
                for blk in range(NRT):           # 16 blocks of 8 groups
                    stage = stg.tile([128, 8, OJ], BF16, tag="stage")
                    # batched loads: x_bd per 4 blocks, W2 per half-block
                    if blk % 2 == 0:
                        xg = xp.tile([128, 16, 128], BF16, tag="xg")
                        nc.sync.dma_start(xg[:], xbd_v[blk // 2])
                    for gg in range(8):
                        if gg % 4 == 0:
                            wg = wp.tile([128, 4, OJ], BF16, tag="wg")
                            eng = nc.scalar if (blk % 2 == 0) else nc.sync
                            eng.dma_start(wg[:], w2_v[blk * 2 + gg // 4])
                        if gg % 2 == 0:
                            ps = bps.tile([128, 2, OJ], F32, tag="ps")
                        nc.tensor.matmul(
                            ps[:, gg % 2, :], xg[:, (blk % 2) * 8 + gg, :], wg[:, gg % 4, :],
                            start=True, stop=True,
                        )
                        if gg % 2 == 1:
                            # paired cast-copy (2 banks), split Scalar/Vector
                            if gg % 4 == 1:
                                nc.scalar.copy(stage[:, gg - 1:gg + 1, :], ps[:])
                            else:
                                nc.vector.tensor_copy(
                                    stage[:, gg - 1:gg + 1, :], ps[:]
                                )
                    # transpose: stage[p=(isub*8+b), gg, oj] ->
                    #            uhat[q=gg*16+isub, blk, b, oj]
                    for gg in range(8):
                        eng = (nc.sync, nc.scalar, nc.gpsimd)[(blk * 8 + gg) % 3]
                        eng.dma_start(
                            uhat[gg * 16:(gg + 1) * 16, blk, :, :],
                            stage[:, gg, :],
                        )

                # ---------------- routing ----------------
                if n_routing == 0:
                    for b in range(BC):
                        nc.gpsimd.dma_start(
                            out_d.ap()[b],
                            uhat[0:OUT_CAPS, 0, b, 0:OUT_DIM],
                        )
                for r in range(n_routing):
                    for b in range(BC):
                        g_ps = gp.tile([OUT_CAPS, OJ], F32, tag="g_ps")
                        for rt in range(NRT):
                            lhsT = (
                                cunif[:]
                                if r == 0
                                else chat[:, rt, b, :]
                            )
                            nc.tensor.matmul(
                                g_ps[:],
                                lhsT,
                                uhat[:, rt, b, :],
                                start=(rt == 0),
                                stop=(rt == NRT - 1),
                            )
                        mprod = sm.tile([OUT_CAPS, OJ], BF16, tag="mprod")
                        nc.vector.tensor_mul(mprod[:], g_ps[:], mask[:])
                        s_b = sm.tile([OUT_CAPS, OUT_DIM], F32, tag="s_b")
                        nc.vector.tensor_reduce(
                            s_b[:],
                            mprod[:].rearrange("p (o j) -> p j o", o=OUT_CAPS),
                            axis=AX,
                            op=add,
                        )
                        # squash: v = s * s2 / ((1+s2) * sqrt(s2+eps))
                        sq = sm.tile([OUT_CAPS, OUT_DIM], F32, tag="sq")
                        s2 = sm.tile([OUT_CAPS, 1], F32, tag="s2")
                        nc.vector.tensor_mul(sq[:], s_b[:], s_b[:])
                        nc.vector.tensor_reduce(s2[:], sq[:], axis=AX, op=add)
                        t1 = sm.tile([OUT_CAPS, 1], F32, tag="t1")
                        nc.gpsimd.tensor_scalar_add(t1[:], s2[:], 1.0)
                        nc.vector.reciprocal(t1[:], t1[:])
                        t2 = sm.tile([OUT_CAPS, 1], F32, tag="t2")
                        nc.gpsimd.tensor_scalar_add(t2[:], s2[:], EPS)
                        nc.scalar.activation(t2[:], t2[:], Sqrt)
                        nc.vector.reciprocal(t2[:], t2[:])
                        nc.gpsimd.tensor_mul(t1[:], t1[:], t2[:])
                        nc.gpsimd.tensor_mul(t1[:], t1[:], s2[:])
                        v_b = sm.tile([OUT_CAPS, OUT_DIM], F32, tag="v_b")
                        nc.vector.tensor_scalar_mul(v_b[:], s_b[:], t1[:])
                        if r < n_routing - 1 and do_update:
                            # broadcast v_b to all 128 partitions (bf16)
                            vb16 = sm.tile([OUT_CAPS, OUT_DIM], BF16, tag="vb16")
                            nc.scalar.copy(vb16[:], v_b[:])
                            vrow = sm.tile([1, OJ], BF16, tag="vrow")
                            nc.sync.dma_start(vrow[:], vb16[:])
                            vps = vp.tile([128, OJ], F32, tag="vps")
                            nc.tensor.matmul(
                                vps[:], ones[:], vrow[:], start=True, stop=True
                            )
                            nc.scalar.copy(vbc[:, b, :], vps[:])
                        elif r == n_routing - 1:
                            nc.sync.dma_start(out_d.ap()[b], v_b[:])

                    if r < n_routing - 1 and do_update:
                        # logit update + softmax, per rt tile
                        for rt in range(NRT):
                            u4 = uhat[:, rt, :, :].rearrange(
                                "p b (o j) -> p b o j", o=OUT_CAPS
                            )
                            v4 = vbc[:].rearrange("p b (o j) -> p b o j", o=OUT_CAPS)
                            prod = tp.tile([128, BC, OUT_CAPS, 16], BF16, tag="prod")
                            nc.vector.tensor_mul(prod[:], u4, v4)
                            tr1 = tp.tile([128, BC, OUT_CAPS, 8], BF16, tag="tr1")
                            nc.vector.tensor_tensor(
                                tr1[:], prod[:, :, :, 0:8], prod[:, :, :, 8:16], op=add
                            )
                            tr2 = tp.tile([128, BC, OUT_CAPS, 4], BF16, tag="tr2")
                            nc.vector.tensor_tensor(
                                tr2[:], tr1[:, :, :, 0:4], tr1[:, :, :, 4:8], op=add
                            )
                            tr3 = tp.tile([128, BC, OUT_CAPS, 2], BF16, tag="tr3")
                            nc.vector.tensor_tensor(
                                tr3[:], tr2[:, :, :, 0:2], tr2[:, :, :, 2:4], op=add
                            )
                            t3a = tr3[:, :, :, 0:1].rearrange("p b o one -> p b (o one)")
                            t3b = tr3[:, :, :, 1:2].rearrange("p b o one -> p b (o one)")
                            if r == 0:
                                nc.vector.tensor_tensor(
                                    bij[:, rt, :, :], t3a, t3b, op=add
                                )
                            else:
                                upd = tp.tile([128, BC, OUT_CAPS], BF16, tag="upd")
                                nc.vector.tensor_tensor(upd[:], t3a, t3b, op=add)
                                nc.vector.tensor_add(
                                    bij[:, rt, :, :], bij[:, rt, :, :], upd[:]
                                )
                            # softmax over o -> chat
                            ex = sm.tile([128, BC, OUT_CAPS], BF16, tag="ex")
                            nc.scalar.activation(ex[:], bij[:, rt, :, :], Exp)
                            zr = sm.tile([128, BC], F32, tag="zr")
                            nc.vector.tensor_reduce(zr[:], ex[:], axis=AX, op=add)
                            nc.vector.reciprocal(zr[:], zr[:])
                            for b in range(BC):
                                nc.vector.tensor_scalar_mul(
                                    chat[:, rt, b, :], ex[:, b, :], zr[:, b:b + 1]
                                )
    nc.compile()
    return nc


def _prep_inputs(x: np.ndarray, W: np.ndarray):
    W0 = np.ascontiguousarray(W.reshape(IN_CAPS, OUT_CAPS, OUT_DIM, IN_DIM))
    # W2[g, isub*8+d, oj] = W0[g*16+isub, oj, d]
    w2 = (
        W0.reshape(NG, 16, OJ, IN_DIM)
        .transpose(0, 1, 3, 2)
        .reshape(NG, 128, OJ)
        .astype(ml_dtypes.bfloat16)
    )
    mask = np.zeros((OUT_CAPS, OJ), np.float32)
    for o in range(OUT_CAPS):
        mask[o, o * OUT_DIM:(o + 1) * OUT_DIM] = 1.0
    ones = np.ones((1, 128), ml_dtypes.bfloat16)

    in_maps = []
    idx = np.arange(16)
    for c in range(N_CORES):
        xc = x[c * BC:(c + 1) * BC]          # [8, 2048, 8]
        # A[g, isub, d, b] = xc[b, g*16+isub, d]
        A = xc.transpose(1, 2, 0).reshape(NG, 16, IN_DIM, BC)
        xbd_v = np.zeros((NG, 16, IN_DIM, 16, BC), np.float32)
        xbd_v[:, idx, :, idx, :] = A.transpose(1, 0, 2, 3)
        xbd = xbd_v.reshape(NG, 128, 128).astype(ml_dtypes.bfloat16)
        in_maps.append({"xbd": xbd, "w2": w2, "mask": mask, "ones": ones})
    return in_maps


def kernel(x: np.ndarray, W: np.ndarray) -> np.ndarray:
    x = np.asarray(x, dtype=np.float32)
    W = np.asarray(W, dtype=np.float32)
    if "nc" not in _CACHE:
        _CACHE["nc"] = build_nc()
    nc = _CACHE["nc"]
    in_maps = _prep_inputs(x, W)
    res = bass_utils.run_bass_kernel_spmd(nc, in_maps, core_ids=list(range(N_CORES)))
    out = np.concatenate([res.results[c]["vout"] for c in range(N_CORES)], axis=0)
    return out.astype(np.float32)


if __name__ == "__main__":
    xt = np.random.randn(B, IN_CAPS, IN_DIM).astype(np.float32)
    Wt = (np.random.randn(1, IN_CAPS, OUT_CAPS, OUT_DIM, IN_DIM) * 0.01).astype(
        np.float32
    )
    print(kernel(xt, Wt).shape)
